# revision 4
# baseline (speedup 1.0000x reference)
"""Trainium2 Bass kernel for LLMGRU4rec.

Strategy: data-parallel over batch (B=128 -> 16 rows/core on 8 cores).
Everything on-device is feature-major ("transposed"): activations live as
[H on partitions, tokens on free dim], so the 2x200-step GRU scan needs no
transposes. MLP embedding phases are K-accumulated bf16 matmuls; the scan
merges the precomputed input-side gate pre-activations into PSUM via
identity matmuls and uses spanning access patterns to keep per-step
instruction count low.
"""

import os
import numpy as np
import ml_dtypes

import concourse.bass as bass
import concourse.mybir as mybir
from concourse.tile import TileContext
from concourse.vector_clock import ScopedClock
from concourse.bass_utils import run_bass_kernel_spmd
from concourse.masks import make_identity

F32 = mybir.dt.float32
BF = mybir.dt.bfloat16
AF = mybir.ActivationFunctionType
OP = mybir.AluOpType

B, T, L, H = 128, 200, 4096, 256
NC = 8
BL = B // NC            # 16 batch rows per core
TOK = BL * T            # 3200 tokens per core
KL = L // 128           # 32 k-chunks for the LLM dim
NBLK = [(i * 512, 512) for i in range(TOK // 512)]
if TOK % 512:
    NBLK.append((TOK - TOK % 512, TOK % 512))


class _SplitDrainTileContext(TileContext):
    """This walrus build caps sync-waits per CTRL instruction; split the
    exit-drain's waits one-per-nofuse-nop."""

    def _drain_and_barrier(self, tick_clock, wait_clock):
        nc = self.nc
        drain_inst = nc.sync.drain(fusable=False)
        wait_clock.add_sem_waits(
            drain_inst.ins, ScopedClock({None: tick_clock.global_clock})
        )
        si = drain_inst.ins.sync_info
        waits = list(si.on_wait) if si is not None else []
        if len(waits) > 1:
            drain_inst.ins.sync_info = mybir.SyncInfo(
                on_wait=[waits[0]], on_update=list(si.on_update)
            )
            for wv in waits[1:]:
                extra = nc.sync.nop(nofuse=True, hint="split_wait")
                extra.ins.sync_info = mybir.SyncInfo(on_wait=[wv], on_update=[])
        nc.all_engine_barrier()
        assert self.sems is not None
        popped = nc._tile_sem_poison_stack.pop()
        assert popped is self._sem_poison
        nc.clear_and_free_semaphores(list(self.sems.allocated().values()))
        nc.all_engine_barrier()


def _mlp3(nc, psA, xsp, evac, x_dram, fcW, w1, w2, bfc, b1, b2, E1, E2,
          out_bf=None, out_dram=None):
    """3-layer MLP, feature-major output. x_dram: [KL,128,TOK] bf16.
    Final layer -> out_bf (bf16 SBUF tile) or f32 evac -> out_dram."""
    for c0, w in NBLK:
        ps = [psA.tile([128, 512], F32, tag="psA", name="psmm") for _ in range(2)]
        for k in range(KL):
            xs = xsp.tile([128, 512], BF, tag="xs")
            nc.sync.dma_start(xs[:, 0:w], x_dram[k, :, c0:c0 + w])
            for m2 in range(2):
                nc.tensor.matmul(ps[m2][:, 0:w],
                                 fcW[:, k, m2 * 128:(m2 + 1) * 128],
                                 xs[:, 0:w], start=(k == 0), stop=(k == KL - 1))
        for m2 in range(2):
            nc.scalar.activation(E1[:, m2, c0:c0 + w], ps[m2][:, 0:w],
                                 AF.Gelu, bias=bfc[:, m2:m2 + 1])
    for wmat, bias, src, dst in ((w1, b1, E1, E2),):
        for c0, w in NBLK:
            for m2 in range(2):
                ps = psA.tile([128, 512], F32, tag="psA")
                for k2 in range(2):
                    nc.tensor.matmul(ps[:, 0:w],
                                     wmat[:, k2, m2 * 128:(m2 + 1) * 128],
                                     src[:, k2, c0:c0 + w],
                                     start=(k2 == 0), stop=(k2 == 1))
                nc.scalar.activation(dst[:, m2, c0:c0 + w], ps[:, 0:w],
                                     AF.Gelu, bias=bias[:, m2:m2 + 1])
    for c0, w in NBLK:
        for m2 in range(2):
            ps = psA.tile([128, 512], F32, tag="psA")
            for k2 in range(2):
                nc.tensor.matmul(ps[:, 0:w],
                                 w2[:, k2, m2 * 128:(m2 + 1) * 128],
                                 E2[:, k2, c0:c0 + w],
                                 start=(k2 == 0), stop=(k2 == 1))
            if out_bf is not None:
                nc.scalar.activation(out_bf[:, m2, c0:c0 + w], ps[:, 0:w],
                                     AF.Gelu, bias=b2[:, m2:m2 + 1])
            else:
                ev = evac.tile([128, 512], F32, tag="ev")
                nc.scalar.activation(ev[:, 0:w], ps[:, 0:w],
                                     AF.Gelu, bias=b2[:, m2:m2 + 1])
                nc.sync.dma_start(out_dram[m2, :, c0:c0 + w], ev[:, 0:w])


def _gi_phase(nc, psA, src, wih, bgi, gi):
    """gi[:,g,:] = wih_g^T-major matmul over src + per-feature bias."""
    for c0, w in NBLK:
        for g in range(6):
            ps = psA.tile([128, 512], F32, tag="psA")
            for k2 in range(2):
                nc.tensor.matmul(ps[:, 0:w],
                                 wih[:, k2, g * 128:(g + 1) * 128],
                                 src[:, k2, c0:c0 + w],
                                 start=(k2 == 0), stop=(k2 == 1))
            nc.vector.tensor_scalar(gi[:, g, c0:c0 + w], ps[:, 0:w],
                                    bgi[:, g:g + 1], None, op0=OP.add)


def _scan(nc, psS, scr, gi, whh, bhn, ident, mexp, hc, hseq):
    nc.vector.memset(hc, 0.0)
    for t in range(T):
        a, b = t * BL, (t + 1) * BL
        ps = psS.tile([128, 6, 512], F32, tag="psS")
        for g in range(6):
            for k2 in range(2):
                nc.tensor.matmul(ps[:, g, 0:BL],
                                 whh[:, k2, g * 128:(g + 1) * 128],
                                 hc[:, k2, :], start=(k2 == 0), stop=False)
        for g in range(4):
            nc.tensor.matmul(ps[:, g, 0:BL], ident, gi[:, g, a:b],
                             start=False, stop=True)
        for g in (4, 5):
            nc.tensor.matmul(ps[:, g, 0:BL], ident, bhn[:, g - 4, :],
                             start=False, stop=True)
        rz = scr.tile([128, 4, BL], BF, tag="rz")
        nc.scalar.activation(rz, ps[:, 0:4, 0:BL], AF.Sigmoid)
        u = scr.tile([128, 2, BL], BF, tag="u")
        nc.vector.tensor_tensor(u, ps[:, 4:6, 0:BL], rz[:, 0:2, :], op=OP.mult)
        q = scr.tile([128, 2, BL], BF, tag="q")
        nc.vector.tensor_tensor(q, u, gi[:, 4:6, a:b], op=OP.add)
        n = scr.tile([128, 2, BL], BF, tag="n")
        nc.scalar.activation(n, q, AF.Tanh)
        d = scr.tile([128, 2, BL], BF, tag="d")
        nc.vector.tensor_tensor(d, hc, n, op=OP.subtract)
        e = scr.tile([128, 2, BL], BF, tag="e")
        nc.vector.tensor_tensor(e, rz[:, 2:4, :], d, op=OP.mult)
        nc.vector.tensor_tensor(hseq[:, :, a:b], e, n, op=OP.add)
        nc.vector.tensor_tensor(hc, hseq[:, :, a:b], mexp[:, t, :], op=OP.mult)


def _split_waits(nc, cap=1):
    """This walrus build rejects instructions carrying more than a couple of
    sync waits. Move excess waits onto injected same-engine nops placed just
    before the instruction (same engine stream => ordering preserved)."""
    nid = 0
    for bb in nc.m.functions[0].blocks:
        newlist = []
        changed = False
        for inst in bb.instructions:
            si = inst.sync_info
            waits = list(si.on_wait) if si is not None else []
            if len(waits) > cap:
                changed = True
                ups = list(si.on_update)
                excess = waits[:-cap]
                for i in range(0, len(excess), cap):
                    nop = mybir.InstNoOp(name=f"I-splitw-{nid}", ins=[], outs=[])
                    nid += 1
                    nop.engine = inst.engine
                    nop.sync_info = mybir.SyncInfo(
                        on_wait=excess[i:i + cap], on_update=[])
                    newlist.append(nop)
                inst.sync_info = mybir.SyncInfo(on_wait=waits[-cap:],
                                                on_update=ups)
            newlist.append(inst)
        if changed:
            bb.instructions = newlist


def _build():
    nc = bass.Bass()
    d = {}
    for nm in ("xT", "nT"):
        d[nm] = nc.dram_tensor(nm, [KL, 128, TOK], BF, kind="ExternalInput")
    d["fcW"] = nc.dram_tensor("fcW", [128, KL, 256], BF, kind="ExternalInput")
    for nm in ("w1", "w2", "h2o"):
        d[nm] = nc.dram_tensor(nm, [128, 2, 256], BF, kind="ExternalInput")
    for nm in ("wih0", "whh0", "wih1", "whh1"):
        d[nm] = nc.dram_tensor(nm, [128, 2, 768], BF, kind="ExternalInput")
    for nm in ("bfc", "b1", "b2", "bh2o"):
        d[nm] = nc.dram_tensor(nm, [128, 2], F32, kind="ExternalInput")
    for nm in ("bgi0", "bgi1"):
        d[nm] = nc.dram_tensor(nm, [128, 6], F32, kind="ExternalInput")
    for nm in ("bhn0", "bhn1"):
        d[nm] = nc.dram_tensor(nm, [128, 2, BL], BF, kind="ExternalInput")
    d["mexp"] = nc.dram_tensor("mexp", [128, T, 2 * BL], BF, kind="ExternalInput")
    Eo = nc.dram_tensor("Eo", [2, 128, TOK], F32, kind="ExternalOutput")
    No = nc.dram_tensor("No", [2, 128, TOK], F32, kind="ExternalOutput")
    Lo = nc.dram_tensor("Lo", [2, 128, TOK], F32, kind="ExternalOutput")

    with _SplitDrainTileContext(nc) as tc:
        with tc.tile_pool(name="persist", bufs=1) as pp:
            fcW = pp.tile([128, KL, 256], BF, tag="fcW")
            nc.sync.dma_start(fcW, d["fcW"][:])
            sb = {}
            for nm in ("w1", "w2", "h2o"):
                sb[nm] = pp.tile([128, 2, 256], BF, tag=nm, name=nm)
                nc.sync.dma_start(sb[nm], d[nm][:])
            for nm in ("wih0", "whh0", "wih1", "whh1"):
                sb[nm] = pp.tile([128, 2, 768], BF, tag=nm, name=nm)
                nc.sync.dma_start(sb[nm], d[nm][:])
            for nm in ("bfc", "b1", "b2", "bh2o"):
                sb[nm] = pp.tile([128, 2], F32, tag=nm, name=nm)
                nc.sync.dma_start(sb[nm], d[nm][:])
            for nm in ("bgi0", "bgi1"):
                sb[nm] = pp.tile([128, 6], F32, tag=nm, name=nm)
                nc.sync.dma_start(sb[nm], d[nm][:])
            for nm in ("bhn0", "bhn1"):
                sb[nm] = pp.tile([128, 2, BL], BF, tag=nm, name=nm)
                nc.sync.dma_start(sb[nm], d[nm][:])
            mexp = pp.tile([128, T, 2 * BL], BF, tag="mexp")
            nc.sync.dma_start(mexp, d["mexp"][:])
            ident = pp.tile([128, 128], BF, tag="ident")
            make_identity(nc, ident)
            E = pp.tile([128, 2, TOK], BF, tag="E")
            gi = pp.tile([128, 6, TOK], BF, tag="gi")
            h0s = pp.tile([128, 2, TOK], BF, tag="h0s")
            h1s = pp.tile([128, 2, TOK], BF, tag="h1s")
            hc = pp.tile([128, 2, BL], BF, tag="hc")

            with tc.tile_pool(name="phA", bufs=1) as pa, \
                 tc.tile_pool(name="psA", bufs=4, space="PSUM") as psA, \
                 tc.tile_pool(name="xs", bufs=6) as xsp, \
                 tc.tile_pool(name="ev", bufs=4) as evac:
                E1 = pa.tile([128, 2, TOK], BF, tag="E1")
                E2 = pa.tile([128, 2, TOK], BF, tag="E2")
                _mlp3(nc, psA, xsp, evac, d["xT"], fcW, sb["w1"], sb["w2"],
                      sb["bfc"], sb["b1"], sb["b2"], E1, E2, out_bf=E)
                for j in range(2):
                    nc.gpsimd.dma_start(Eo[j], E[:, j, :])
                _mlp3(nc, psA, xsp, evac, d["nT"], fcW, sb["w1"], sb["w2"],
                      sb["bfc"], sb["b1"], sb["b2"], E1, E2, out_dram=No)
                _gi_phase(nc, psA, E, sb["wih0"], sb["bgi0"], gi)

            with tc.tile_pool(name="psS", bufs=1, space="PSUM") as psS, \
                 tc.tile_pool(name="scr", bufs=3) as scr:
                _scan(nc, psS, scr, gi, sb["whh0"], sb["bhn0"], ident,
                      mexp, hc, h0s)

            with tc.tile_pool(name="psB", bufs=4, space="PSUM") as psB:
                _gi_phase(nc, psB, h0s, sb["wih1"], sb["bgi1"], gi)

            with tc.tile_pool(name="psS2", bufs=1, space="PSUM") as psS2, \
                 tc.tile_pool(name="scr2", bufs=3) as scr2:
                _scan(nc, psS2, scr2, gi, sb["whh1"], sb["bhn1"], ident,
                      mexp, hc, h1s)

            with tc.tile_pool(name="psF", bufs=4, space="PSUM") as psF, \
                 tc.tile_pool(name="evF", bufs=4) as evF:
                for c0, w in NBLK:
                    for m2 in range(2):
                        ps = psF.tile([128, 512], F32, tag="psF")
                        for k2 in range(2):
                            nc.tensor.matmul(ps[:, 0:w],
                                             sb["h2o"][:, k2, m2 * 128:(m2 + 1) * 128],
                                             h1s[:, k2, c0:c0 + w],
                                             start=(k2 == 0), stop=(k2 == 1))
                        ev = evF.tile([128, 512], F32, tag="evL")
                        nc.scalar.activation(ev[:, 0:w], ps[:, 0:w], AF.Tanh,
                                             bias=sb["bh2o"][:, m2:m2 + 1])
                        nc.sync.dma_start(Lo[m2, :, c0:c0 + w], ev[:, 0:w])
    _split_waits(nc)
    return nc


_NC_CACHE = {}


def _get_nc():
    if "nc" not in _NC_CACHE:
        _NC_CACHE["nc"] = _build()
    return _NC_CACHE["nc"]


def _fm(a):
    """[tok, F] f32 -> [F/128, 128, tok] bf16 feature-major chunks."""
    tok, F = a.shape
    return np.ascontiguousarray(
        a.astype(ml_dtypes.bfloat16).reshape(tok, F // 128, 128).transpose(1, 2, 0))


def _pack_w(wT):
    """[K, M] -> [128, K/128, M] partition-major."""
    K, M = wT.shape
    return np.ascontiguousarray(
        wT.astype(ml_dtypes.bfloat16).reshape(K // 128, 128, M).transpose(1, 0, 2))


def _pack_b(b):
    """[F] f32 -> [128, F/128]."""
    return np.ascontiguousarray(
        np.asarray(b, np.float32).reshape(-1, 128).T)


def kernel(interaction_list, interaction_mask, neg_list,
           fc_W, fc_b, mlp_W1, mlp_b1, mlp_W2, mlp_b2,
           gru_Wih0, gru_Whh0, gru_bih0, gru_bhh0,
           gru_Wih1, gru_Whh1, gru_bih1, gru_bhh1,
           h2o_W, h2o_b, _collect=None):
    nc = _get_nc()
    fc_W = np.asarray(fc_W, np.float32)
    shared = {
        "fcW": _pack_w(fc_W.T),
        "w1": _pack_w(np.asarray(mlp_W1, np.float32).T),
        "w2": _pack_w(np.asarray(mlp_W2, np.float32).T),
        "h2o": _pack_w(np.asarray(h2o_W, np.float32).T),
        "wih0": _pack_w(np.asarray(gru_Wih0, np.float32).T),
        "whh0": _pack_w(np.asarray(gru_Whh0, np.float32).T),
        "wih1": _pack_w(np.asarray(gru_Wih1, np.float32).T),
        "whh1": _pack_w(np.asarray(gru_Whh1, np.float32).T),
        "bfc": _pack_b(fc_b), "b1": _pack_b(mlp_b1), "b2": _pack_b(mlp_b2),
        "bh2o": _pack_b(h2o_b),
    }
    for i, (bih, bhh) in enumerate(((gru_bih0, gru_bhh0), (gru_bih1, gru_bhh1))):
        bih = np.asarray(bih, np.float32)
        bhh = np.asarray(bhh, np.float32)
        bg = bih.copy()
        bg[:512] += bhh[:512]
        shared[f"bgi{i}"] = _pack_b(bg)
        bhn = bhh[512:].reshape(2, 128)
        shared[f"bhn{i}"] = np.ascontiguousarray(
            np.broadcast_to(bhn.transpose(1, 0)[:, :, None], (128, 2, BL))
        ).astype(ml_dtypes.bfloat16)

    X = np.asarray(interaction_list, np.float32)
    Ng = np.asarray(neg_list, np.float32)
    M = np.asarray(interaction_mask)
    in_maps = []
    for c in range(NC):
        s = slice(c * BL, (c + 1) * BL)
        # token order: col = t*BL + b  (t-major)
        xt = X[s].transpose(1, 0, 2).reshape(TOK, L)      # [t,b,L] -> flat
        ng = Ng[s].transpose(1, 0, 2).reshape(TOK, L)
        m = (M[s] != 0).astype(np.float32)                # [BL, T]
        mex = np.broadcast_to(
            np.tile(m.T.astype(ml_dtypes.bfloat16), 2)[None, :, :],
            (128, T, 2 * BL))
        in_maps.append(dict(shared,
                            xT=_fm(xt), nT=_fm(ng),
                            mexp=np.ascontiguousarray(mex)))

    trace = bool(int(os.environ.get("GRU_TRACE", "0")))
    res = run_bass_kernel_spmd(nc, in_maps, core_ids=list(range(NC)),
                               trace=trace)
    if _collect is not None:
        _collect.append(res)

    E_full = np.empty((B, T, H), np.float32)
    N_full = np.empty((B, T, H), np.float32)
    L_full = np.empty((B, T, H), np.float32)
    for c in range(NC):
        s = slice(c * BL, (c + 1) * BL)
        for out, dst in (("Eo", E_full), ("No", N_full), ("Lo", L_full)):
            v = res.results[c][out]                        # [2,128,TOK]
            dst[s] = v.reshape(2, 128, T, BL).transpose(3, 2, 0, 1).reshape(BL, T, H)
    return (L_full[:, :-1, :], E_full[:, 1:, :], N_full[:, :-1, :],
            np.concatenate((E_full, L_full), axis=2))


# revision 5
# speedup vs baseline: 10323.3059x; 10323.3059x over previous
"""Trainium2 Bass kernel for LLMGRU4rec.

Strategy: data-parallel over batch (B=128 -> 16 rows/core on 8 cores).
Everything on-device is feature-major ("transposed"): activations live as
[H on partitions, tokens on free dim], so the 2x200-step GRU scan needs no
transposes. MLP embedding phases are K-accumulated bf16 matmuls; the scan
merges the precomputed input-side gate pre-activations into PSUM via
identity matmuls and uses spanning access patterns to keep per-step
instruction count low.
"""

import os
import numpy as np
import ml_dtypes

import concourse.bass as bass
import concourse.mybir as mybir
from concourse.tile import TileContext
from concourse.vector_clock import ScopedClock
from concourse.bass_utils import run_bass_kernel_spmd
from concourse.masks import make_identity

F32 = mybir.dt.float32
BF = mybir.dt.bfloat16
AF = mybir.ActivationFunctionType
OP = mybir.AluOpType

B, T, L, H = 128, 200, 4096, 256
NC = 8
BL = B // NC            # 16 batch rows per core
TOK = BL * T            # 3200 tokens per core
KL = L // 128           # 32 k-chunks for the LLM dim
NBLK = [(i * 512, 512) for i in range(TOK // 512)]
if TOK % 512:
    NBLK.append((TOK - TOK % 512, TOK % 512))


class _SplitDrainTileContext(TileContext):
    """This walrus build caps sync-waits per CTRL instruction; split the
    exit-drain's waits one-per-nofuse-nop."""

    def _drain_and_barrier(self, tick_clock, wait_clock):
        nc = self.nc
        drain_inst = nc.sync.drain(fusable=False)
        wait_clock.add_sem_waits(
            drain_inst.ins, ScopedClock({None: tick_clock.global_clock})
        )
        si = drain_inst.ins.sync_info
        waits = list(si.on_wait) if si is not None else []
        if len(waits) > 1:
            drain_inst.ins.sync_info = mybir.SyncInfo(
                on_wait=[waits[0]], on_update=list(si.on_update)
            )
            for wv in waits[1:]:
                extra = nc.sync.nop(nofuse=True, hint="split_wait")
                extra.ins.sync_info = mybir.SyncInfo(on_wait=[wv], on_update=[])
        nc.all_engine_barrier()
        assert self.sems is not None
        popped = nc._tile_sem_poison_stack.pop()
        assert popped is self._sem_poison
        nc.clear_and_free_semaphores(list(self.sems.allocated().values()))
        nc.all_engine_barrier()


def _mlp3(nc, psA, xsp, evac, x_dram, fcW, w1, w2, bfc, b1, b2, E1, E2,
          out_bf=None, out_dram=None):
    """3-layer MLP, feature-major output. x_dram: [KL,128,TOK] bf16.
    Final layer -> out_bf (bf16 SBUF tile) or f32 evac -> out_dram."""
    for c0, w in NBLK:
        ps = [psA.tile([128, 512], F32, tag="psA", name="psmm") for _ in range(2)]
        for k in range(KL):
            xs = xsp.tile([128, 512], BF, tag="xs")
            nc.sync.dma_start(xs[:, 0:w], x_dram[k, :, c0:c0 + w])
            for m2 in range(2):
                nc.tensor.matmul(ps[m2][:, 0:w],
                                 fcW[:, k, m2 * 128:(m2 + 1) * 128],
                                 xs[:, 0:w], start=(k == 0), stop=(k == KL - 1))
        for m2 in range(2):
            nc.scalar.activation(E1[:, m2, c0:c0 + w], ps[m2][:, 0:w],
                                 AF.Gelu, bias=bfc[:, m2:m2 + 1])
    for wmat, bias, src, dst in ((w1, b1, E1, E2),):
        for c0, w in NBLK:
            for m2 in range(2):
                ps = psA.tile([128, 512], F32, tag="psA")
                for k2 in range(2):
                    nc.tensor.matmul(ps[:, 0:w],
                                     wmat[:, k2, m2 * 128:(m2 + 1) * 128],
                                     src[:, k2, c0:c0 + w],
                                     start=(k2 == 0), stop=(k2 == 1))
                nc.scalar.activation(dst[:, m2, c0:c0 + w], ps[:, 0:w],
                                     AF.Gelu, bias=bias[:, m2:m2 + 1])
    for c0, w in NBLK:
        for m2 in range(2):
            ps = psA.tile([128, 512], F32, tag="psA")
            for k2 in range(2):
                nc.tensor.matmul(ps[:, 0:w],
                                 w2[:, k2, m2 * 128:(m2 + 1) * 128],
                                 E2[:, k2, c0:c0 + w],
                                 start=(k2 == 0), stop=(k2 == 1))
            if out_bf is not None:
                nc.scalar.activation(out_bf[:, m2, c0:c0 + w], ps[:, 0:w],
                                     AF.Gelu, bias=b2[:, m2:m2 + 1])
            else:
                ev = evac.tile([128, 512], F32, tag="ev")
                nc.scalar.activation(ev[:, 0:w], ps[:, 0:w],
                                     AF.Gelu, bias=b2[:, m2:m2 + 1])
                nc.sync.dma_start(out_dram[m2, :, c0:c0 + w], ev[:, 0:w])


def _gi_phase(nc, psA, src, wih, bgi, gi):
    """gi[:,g,:] = wih_g^T-major matmul over src + per-feature bias."""
    for c0, w in NBLK:
        for g in range(6):
            ps = psA.tile([128, 512], F32, tag="psA")
            for k2 in range(2):
                nc.tensor.matmul(ps[:, 0:w],
                                 wih[:, k2, g * 128:(g + 1) * 128],
                                 src[:, k2, c0:c0 + w],
                                 start=(k2 == 0), stop=(k2 == 1))
            nc.vector.tensor_scalar(gi[:, g, c0:c0 + w], ps[:, 0:w],
                                    bgi[:, g:g + 1], None, op0=OP.add)


def _scan(nc, psS, scr, gi, whh, bhn, ident, mexp, hc, hseq):
    nc.vector.memset(hc, 0.0)
    for t in range(T):
        a, b = t * BL, (t + 1) * BL
        ps = psS.tile([128, 6, 512], F32, tag="psS")
        # hc-independent accumulands first: they can run during the previous
        # step's elementwise tail (start=True clears the bank's has_written).
        for g in range(4):
            nc.tensor.matmul(ps[:, g, 0:BL], ident, gi[:, g, a:b],
                             start=True, stop=False)
        for g in (4, 5):
            nc.tensor.matmul(ps[:, g, 0:BL], ident, bhn[:, g - 4, :],
                             start=True, stop=False)
        for g in range(6):
            for k2 in range(2):
                nc.tensor.matmul(ps[:, g, 0:BL],
                                 whh[:, k2, g * 128:(g + 1) * 128],
                                 hc[:, k2, :], start=False, stop=(k2 == 1))
        rz = scr.tile([128, 4, BL], BF, tag="rz")
        nc.scalar.activation(rz, ps[:, 0:4, 0:BL], AF.Sigmoid)
        u = scr.tile([128, 2, BL], BF, tag="u")
        nc.vector.tensor_tensor(u, ps[:, 4:6, 0:BL], rz[:, 0:2, :], op=OP.mult)
        q = scr.tile([128, 2, BL], BF, tag="q")
        nc.vector.tensor_tensor(q, u, gi[:, 4:6, a:b], op=OP.add)
        n = scr.tile([128, 2, BL], BF, tag="n")
        nc.scalar.activation(n, q, AF.Tanh)
        d = scr.tile([128, 2, BL], BF, tag="d")
        nc.vector.tensor_tensor(d, hc, n, op=OP.subtract)
        e = scr.tile([128, 2, BL], BF, tag="e")
        nc.vector.tensor_tensor(e, rz[:, 2:4, :], d, op=OP.mult)
        nc.vector.tensor_tensor(hseq[:, :, a:b], e, n, op=OP.add)
        nc.vector.tensor_tensor(hc, hseq[:, :, a:b], mexp[:, t, :], op=OP.mult)


def _split_waits(nc, cap=1):
    """This walrus build rejects instructions carrying more than a couple of
    sync waits. Move excess waits onto injected same-engine nops placed just
    before the instruction (same engine stream => ordering preserved)."""
    nid = 0
    for bb in nc.m.functions[0].blocks:
        newlist = []
        changed = False
        for inst in bb.instructions:
            si = inst.sync_info
            waits = list(si.on_wait) if si is not None else []
            if len(waits) > cap:
                changed = True
                ups = list(si.on_update)
                excess = waits[:-cap]
                for i in range(0, len(excess), cap):
                    nop = mybir.InstNoOp(name=f"I-splitw-{nid}", ins=[], outs=[])
                    nid += 1
                    nop.engine = inst.engine
                    nop.sync_info = mybir.SyncInfo(
                        on_wait=excess[i:i + cap], on_update=[])
                    newlist.append(nop)
                inst.sync_info = mybir.SyncInfo(on_wait=waits[-cap:],
                                                on_update=ups)
            newlist.append(inst)
        if changed:
            bb.instructions = newlist


def _build():
    nc = bass.Bass()
    d = {}
    for nm in ("xT", "nT"):
        d[nm] = nc.dram_tensor(nm, [KL, 128, TOK], BF, kind="ExternalInput")
    d["fcW"] = nc.dram_tensor("fcW", [128, KL, 256], BF, kind="ExternalInput")
    for nm in ("w1", "w2", "h2o"):
        d[nm] = nc.dram_tensor(nm, [128, 2, 256], BF, kind="ExternalInput")
    for nm in ("wih0", "whh0", "wih1", "whh1"):
        d[nm] = nc.dram_tensor(nm, [128, 2, 768], BF, kind="ExternalInput")
    for nm in ("bfc", "b1", "b2", "bh2o"):
        d[nm] = nc.dram_tensor(nm, [128, 2], F32, kind="ExternalInput")
    for nm in ("bgi0", "bgi1"):
        d[nm] = nc.dram_tensor(nm, [128, 6], F32, kind="ExternalInput")
    for nm in ("bhn0", "bhn1"):
        d[nm] = nc.dram_tensor(nm, [128, 2, BL], BF, kind="ExternalInput")
    d["mexp"] = nc.dram_tensor("mexp", [128, T, 2 * BL], BF, kind="ExternalInput")
    Eo = nc.dram_tensor("Eo", [2, 128, TOK], F32, kind="ExternalOutput")
    No = nc.dram_tensor("No", [2, 128, TOK], F32, kind="ExternalOutput")
    Lo = nc.dram_tensor("Lo", [2, 128, TOK], F32, kind="ExternalOutput")

    with _SplitDrainTileContext(nc) as tc:
        with tc.tile_pool(name="persist", bufs=1) as pp:
            fcW = pp.tile([128, KL, 256], BF, tag="fcW")
            nc.sync.dma_start(fcW, d["fcW"][:])
            sb = {}
            for nm in ("w1", "w2", "h2o"):
                sb[nm] = pp.tile([128, 2, 256], BF, tag=nm, name=nm)
                nc.sync.dma_start(sb[nm], d[nm][:])
            for nm in ("wih0", "whh0", "wih1", "whh1"):
                sb[nm] = pp.tile([128, 2, 768], BF, tag=nm, name=nm)
                nc.sync.dma_start(sb[nm], d[nm][:])
            for nm in ("bfc", "b1", "b2", "bh2o"):
                sb[nm] = pp.tile([128, 2], F32, tag=nm, name=nm)
                nc.sync.dma_start(sb[nm], d[nm][:])
            for nm in ("bgi0", "bgi1"):
                sb[nm] = pp.tile([128, 6], F32, tag=nm, name=nm)
                nc.sync.dma_start(sb[nm], d[nm][:])
            for nm in ("bhn0", "bhn1"):
                sb[nm] = pp.tile([128, 2, BL], BF, tag=nm, name=nm)
                nc.sync.dma_start(sb[nm], d[nm][:])
            mexp = pp.tile([128, T, 2 * BL], BF, tag="mexp")
            nc.sync.dma_start(mexp, d["mexp"][:])
            ident = pp.tile([128, 128], BF, tag="ident")
            make_identity(nc, ident)
            E = pp.tile([128, 2, TOK], BF, tag="E")
            gi = pp.tile([128, 6, TOK], BF, tag="gi")
            h0s = pp.tile([128, 2, TOK], BF, tag="h0s")
            h1s = pp.tile([128, 2, TOK], BF, tag="h1s")
            hc = pp.tile([128, 2, BL], BF, tag="hc")

            with tc.tile_pool(name="phA", bufs=1) as pa, \
                 tc.tile_pool(name="psA", bufs=4, space="PSUM") as psA, \
                 tc.tile_pool(name="xs", bufs=6) as xsp, \
                 tc.tile_pool(name="ev", bufs=4) as evac:
                E1 = pa.tile([128, 2, TOK], BF, tag="E1")
                E2 = pa.tile([128, 2, TOK], BF, tag="E2")
                _mlp3(nc, psA, xsp, evac, d["xT"], fcW, sb["w1"], sb["w2"],
                      sb["bfc"], sb["b1"], sb["b2"], E1, E2, out_bf=E)
                for j in range(2):
                    nc.gpsimd.dma_start(Eo[j], E[:, j, :])
                _mlp3(nc, psA, xsp, evac, d["nT"], fcW, sb["w1"], sb["w2"],
                      sb["bfc"], sb["b1"], sb["b2"], E1, E2, out_dram=No)
                _gi_phase(nc, psA, E, sb["wih0"], sb["bgi0"], gi)

            with tc.tile_pool(name="psS", bufs=1, space="PSUM") as psS, \
                 tc.tile_pool(name="scr", bufs=3) as scr:
                _scan(nc, psS, scr, gi, sb["whh0"], sb["bhn0"], ident,
                      mexp, hc, h0s)

            with tc.tile_pool(name="psB", bufs=4, space="PSUM") as psB:
                _gi_phase(nc, psB, h0s, sb["wih1"], sb["bgi1"], gi)

            with tc.tile_pool(name="psS2", bufs=1, space="PSUM") as psS2, \
                 tc.tile_pool(name="scr2", bufs=3) as scr2:
                _scan(nc, psS2, scr2, gi, sb["whh1"], sb["bhn1"], ident,
                      mexp, hc, h1s)

            with tc.tile_pool(name="psF", bufs=4, space="PSUM") as psF, \
                 tc.tile_pool(name="evF", bufs=4) as evF:
                for c0, w in NBLK:
                    for m2 in range(2):
                        ps = psF.tile([128, 512], F32, tag="psF")
                        for k2 in range(2):
                            nc.tensor.matmul(ps[:, 0:w],
                                             sb["h2o"][:, k2, m2 * 128:(m2 + 1) * 128],
                                             h1s[:, k2, c0:c0 + w],
                                             start=(k2 == 0), stop=(k2 == 1))
                        ev = evF.tile([128, 512], F32, tag="evL")
                        nc.scalar.activation(ev[:, 0:w], ps[:, 0:w], AF.Tanh,
                                             bias=sb["bh2o"][:, m2:m2 + 1])
                        nc.sync.dma_start(Lo[m2, :, c0:c0 + w], ev[:, 0:w])
    _split_waits(nc)
    return nc


_NC_CACHE = {}


def _get_nc():
    if "nc" not in _NC_CACHE:
        _NC_CACHE["nc"] = _build()
    return _NC_CACHE["nc"]


def _fm(a):
    """[tok, F] f32 -> [F/128, 128, tok] bf16 feature-major chunks."""
    tok, F = a.shape
    return np.ascontiguousarray(
        a.astype(ml_dtypes.bfloat16).reshape(tok, F // 128, 128).transpose(1, 2, 0))


def _pack_w(wT):
    """[K, M] -> [128, K/128, M] partition-major."""
    K, M = wT.shape
    return np.ascontiguousarray(
        wT.astype(ml_dtypes.bfloat16).reshape(K // 128, 128, M).transpose(1, 0, 2))


def _pack_b(b):
    """[F] f32 -> [128, F/128]."""
    return np.ascontiguousarray(
        np.asarray(b, np.float32).reshape(-1, 128).T)


def kernel(interaction_list, interaction_mask, neg_list,
           fc_W, fc_b, mlp_W1, mlp_b1, mlp_W2, mlp_b2,
           gru_Wih0, gru_Whh0, gru_bih0, gru_bhh0,
           gru_Wih1, gru_Whh1, gru_bih1, gru_bhh1,
           h2o_W, h2o_b, _collect=None):
    nc = _get_nc()
    fc_W = np.asarray(fc_W, np.float32)
    shared = {
        "fcW": _pack_w(fc_W.T),
        "w1": _pack_w(np.asarray(mlp_W1, np.float32).T),
        "w2": _pack_w(np.asarray(mlp_W2, np.float32).T),
        "h2o": _pack_w(np.asarray(h2o_W, np.float32).T),
        "wih0": _pack_w(np.asarray(gru_Wih0, np.float32).T),
        "whh0": _pack_w(np.asarray(gru_Whh0, np.float32).T),
        "wih1": _pack_w(np.asarray(gru_Wih1, np.float32).T),
        "whh1": _pack_w(np.asarray(gru_Whh1, np.float32).T),
        "bfc": _pack_b(fc_b), "b1": _pack_b(mlp_b1), "b2": _pack_b(mlp_b2),
        "bh2o": _pack_b(h2o_b),
    }
    for i, (bih, bhh) in enumerate(((gru_bih0, gru_bhh0), (gru_bih1, gru_bhh1))):
        bih = np.asarray(bih, np.float32)
        bhh = np.asarray(bhh, np.float32)
        bg = bih.copy()
        bg[:512] += bhh[:512]
        shared[f"bgi{i}"] = _pack_b(bg)
        bhn = bhh[512:].reshape(2, 128)
        shared[f"bhn{i}"] = np.ascontiguousarray(
            np.broadcast_to(bhn.transpose(1, 0)[:, :, None], (128, 2, BL))
        ).astype(ml_dtypes.bfloat16)

    X = np.asarray(interaction_list, np.float32)
    Ng = np.asarray(neg_list, np.float32)
    M = np.asarray(interaction_mask)
    in_maps = []
    for c in range(NC):
        s = slice(c * BL, (c + 1) * BL)
        # token order: col = t*BL + b  (t-major)
        xt = X[s].transpose(1, 0, 2).reshape(TOK, L)      # [t,b,L] -> flat
        ng = Ng[s].transpose(1, 0, 2).reshape(TOK, L)
        m = (M[s] != 0).astype(np.float32)                # [BL, T]
        mex = np.broadcast_to(
            np.tile(m.T.astype(ml_dtypes.bfloat16), 2)[None, :, :],
            (128, T, 2 * BL))
        in_maps.append(dict(shared,
                            xT=_fm(xt), nT=_fm(ng),
                            mexp=np.ascontiguousarray(mex)))

    trace = bool(int(os.environ.get("GRU_TRACE", "0")))
    res = run_bass_kernel_spmd(nc, in_maps, core_ids=list(range(NC)),
                               trace=trace)
    if _collect is not None:
        _collect.append(res)

    E_full = np.empty((B, T, H), np.float32)
    N_full = np.empty((B, T, H), np.float32)
    L_full = np.empty((B, T, H), np.float32)
    for c in range(NC):
        s = slice(c * BL, (c + 1) * BL)
        for out, dst in (("Eo", E_full), ("No", N_full), ("Lo", L_full)):
            v = res.results[c][out]                        # [2,128,TOK]
            dst[s] = v.reshape(2, 128, T, BL).transpose(3, 2, 0, 1).reshape(BL, T, H)
    return (L_full[:, :-1, :], E_full[:, 1:, :], N_full[:, :-1, :],
            np.concatenate((E_full, L_full), axis=2))
